# revision 11
# baseline (speedup 1.0000x reference)
"""Trainium2 Bass kernel for nn_DiffusionPolicyHead (EDM/DDIM sampler head).

Strategy (v2)
-------------
Pure data parallel over 8 NeuronCores (batch 32768 -> 4096/core).

Host algebra identical to v1 (per-step scalars folded; z-substitution
z_{t+1} = z_t + s_t * (h3 @ Wout); layer-0 bias row e'_t moved into the
drain epilogue so the z matmul is a clean K=32).

Device-side changes vs v1:
  * Hidden layers run as fp8e4 DoubleRow matmuls: K=256 in ONE matmul at
    2 fp8 MACs/cell/cycle (~2x bf16 rate). Weights are host-quantized to
    e4m3 at x4 scale with output-aware rounding (coordinate descent
    minimizing ||dW^T H|| over real activation samples) plus a mean
    correction folded into the drain bias. Activations h0..h2 are stored
    fp8 (x1/x4/x16 scale), h3 bf16 (x64), Wout/64 in bf16.
  * Layer-0 z matmul (K=32) runs 2-per-chunk row-tiled (tile_position) so
    two blocks stream concurrently; the out matmul (M=32) runs col-tiled,
    4 blocks into one [128,512] psum, making the z-update a single fused
    DVE op per 4-block group.
  * PSUM tiles are [128,1024] (2 banks); every relu epilogue drains 1024
    columns in one instruction, split ACT/DVE to balance the two engines.

Layout: activations feature-major [feat, batch]; batch 4096 = 4 chunks of
1024 = 8 blocks of 512. z is stored "stacked": group g holds blocks
4g..4g+3 at partition offsets 32b, matching both the row-tiled layer-0
matmul (rhs partition base = tile row) and the col-tiled out matmul
(psum partition base = tile col).
"""

import os
import sys

sys.path.insert(0, "/opt/trn_rl_repo")

import numpy as np
import ml_dtypes

BATCH, STATE_DIM, ACTION_DIM = 32768, 128, 32
HIDDEN, EMBED, N_STEPS = 256, 64, 50
SIGMA_MAX, SIGMA_MIN, RHO = 80.0, 0.001, 7.0
N_CORES = 8
B_CORE = BATCH // N_CORES      # 4096
NB = 512                       # block width
NC = 1024                      # chunk width (2 blocks)
NCHUNKS = B_CORE // NC         # 4
NGROUPS = 2                    # z-stack groups of 4 blocks
P = 128

E4NP = ml_dtypes.float8_e4m3   # IEEE e4m3 (max 240) == TRN FP8_EXP4
BF16NP = ml_dtypes.bfloat16

BETA_W = 4.0                   # hidden weight scale before e4m3 quantization
ALPHA = [1.0, 4.0, 16.0, 64.0]  # stored scale of h0..h3

_cached = {}


def _e4m3_grid():
    u = np.arange(256, dtype=np.uint8).view(E4NP).astype(np.float32)
    u = u[np.isfinite(u)]
    return np.unique(u)


def _optimize_rounding(Ws, H, passes=3):
    """Output-aware rounding of Ws ([K,M], already scaled) to e4m3.

    Coordinate descent over rows k minimizing ||(q-Ws)^T H||_F^2, vectorized
    over output columns. Returns (q_fp8 [K,M], b_corr [M]) where b_corr is
    the residual mean correction in psum units (add to the drain bias).
    """
    grid = _e4m3_grid()
    K, M = Ws.shape
    q = Ws.astype(E4NP).astype(np.float32)
    idx = np.searchsorted(grid, q)
    lower = grid[np.clip(idx - 1, 0, len(grid) - 1)]
    upper = grid[np.clip(idx + 1, 0, len(grid) - 1)]
    alt = np.where(q > Ws, lower, np.where(q < Ws, upper, q)).astype(np.float32)

    R = (q - Ws).T @ H                     # [M, S]
    Hsq = np.einsum("ks,ks->k", H, H)      # [K]
    for _ in range(passes):
        flips = 0
        for k in range(K):
            if Hsq[k] == 0.0:
                continue
            d = alt[k] - q[k]              # [M]
            if not d.any():
                continue
            rh = R @ H[k]                  # [M]
            dobj = 2.0 * d * rh + d * d * Hsq[k]
            take = dobj < -1e-9
            if take.any():
                R[take] += np.outer(d[take], H[k])
                qk = q[k, take].copy()
                q[k, take] = alt[k, take]
                alt[k, take] = qk
                flips += int(take.sum())
        if flips == 0:
            break
    b_corr = -R.mean(axis=1)
    return q.astype(E4NP), b_corr.astype(np.float32)


def _host_tables(W0, b0, Wh, bh, Wout, bout):
    """Fold per-step diffusion constants into weight tables (float64)."""
    W0 = W0.astype(np.float64)
    b0 = b0.astype(np.float64)
    bout = bout.astype(np.float64)
    W0a = W0[:ACTION_DIM]
    W0e = W0[ACTION_DIM : ACTION_DIM + EMBED]
    W0s = W0[ACTION_DIM + EMBED :]

    ramp = np.linspace(0.0, 1.0, N_STEPS)
    min_r, max_r = SIGMA_MIN ** (1.0 / RHO), SIGMA_MAX ** (1.0 / RHO)
    sig = np.concatenate([(max_r + ramp * (min_r - max_r)) ** RHO, np.zeros(1)])

    half = EMBED // 2
    freqs = np.exp(-np.log(10000.0) * np.arange(half, dtype=np.float64) / half)

    sd = 1.0
    g = sig[0]
    beta = np.zeros(ACTION_DIM)
    W0A = np.empty((ACTION_DIM + 1, N_STEPS, HIDDEN), np.float64)
    s_t = np.empty(N_STEPS)
    for t in range(N_STEPS):
        s, sn = sig[t], sig[t + 1]
        var = s * s + sd * sd
        c_in = 1.0 / np.sqrt(var)
        c_skip = sd * sd / var
        c_out = s * sd / np.sqrt(var)
        ratio = sn / s
        a_t = ratio + (1.0 - ratio) * c_skip
        b_t = (1.0 - ratio) * c_out
        ang = np.log(s) * freqs
        emb = np.concatenate([np.sin(ang), np.cos(ang)])
        W0A[:ACTION_DIM, t] = c_in * g * W0a
        W0A[ACTION_DIM, t] = emb @ W0e + b0 + c_in * (beta @ W0a)
        g_next = a_t * g
        beta = a_t * beta + b_t * bout
        s_t[t] = b_t / g_next
        g = g_next
    return dict(
        W0A=np.ascontiguousarray(W0A, np.float32),  # [33, 50, 256]
        W0s=np.ascontiguousarray(W0s, np.float32),  # [128, 256]
        s_t=s_t.astype(np.float32),
        g_final=g,
        beta_final=beta,
    )


def _quantize_hidden(tables, Wh, bh, state_s, noise_s):
    """Quantize the 3 hidden layers to scaled e4m3 with output-aware
    rounding against real activation samples, returning the DoubleRow
    weight layout and the per-layer drain biases (psum units)."""
    W0A = tables["W0A"]
    W0s = tables["W0s"]
    sp = state_s.astype(np.float32) @ W0s           # [S0, 256]
    n_steps = W0A.shape[1]

    def h_pipe(Whq_list):
        """Run the sampler on the sample, mirroring device quantization;
        collect per-layer rhs pools."""
        z = noise_s.astype(np.float32).copy()
        pools = [[], [], []]
        for t in range(n_steps):
            pre0 = sp + z @ W0A[:ACTION_DIM, t] + W0A[ACTION_DIM, t]
            h = np.maximum(pre0, 0.0).astype(E4NP).astype(np.float32)
            for l in range(3):
                pools[l].append(h.T.copy())        # [256, S0] stored units
                pre = h @ Whq_list[l]
                hn = np.maximum(pre + bh[l] * ALPHA[l + 1], 0.0)
                if l < 2:
                    h = hn.astype(E4NP).astype(np.float32)
                else:
                    h = hn.astype(BF16NP).astype(np.float32)
            inner = (h / ALPHA[3]) @ np.asarray(
                tables["Wout_f"], np.float32)
            z = z + tables["s_t"][t] * inner
        return pools

    # pass 1: plain RNE weights to generate activation pools
    rne = [(BETA_W * Wh[l]).astype(np.float32).astype(E4NP).astype(np.float32)
           for l in range(3)]
    pools = h_pipe(rne)

    Whq, bcorr = [], []
    for l in range(3):
        H = np.concatenate(pools[l], axis=1)       # [256, S]
        Ws = (BETA_W * Wh[l]).astype(np.float32)   # [256, 256]
        q, bc = _optimize_rounding(Ws, H)
        Whq.append(q)
        bcorr.append(bc)
    return Whq, bcorr


def _build_program(n_steps):
    import concourse.bacc as bacc
    import concourse.mybir as mybir
    from concourse import tile
    from contextlib import ExitStack

    F32 = mybir.dt.float32
    F32R = mybir.dt.float32r
    BF16 = mybir.dt.bfloat16
    FP8 = mybir.dt.float8e4
    AF = mybir.ActivationFunctionType
    ALU = mybir.AluOpType
    DR = mybir.MatmulPerfMode.DoubleRow

    nc = bacc.Bacc("TRN2", target_bir_lowering=False, debug=False, num_devices=N_CORES)

    state_in = nc.declare_dram_parameter("stateT", [P, B_CORE], F32R, isOutput=False)
    zin_in = nc.declare_dram_parameter("zinit", [NGROUPS, P, NB], F32R, isOutput=False)
    w0s_in = nc.declare_dram_parameter("W0s", [P, HIDDEN], F32R, isOutput=False)
    w0a_in = nc.declare_dram_parameter(
        "W0Arep", [n_steps, P, HIDDEN], F32R, isOutput=False)
    wh_in = nc.declare_dram_parameter("WhDR", [P, 2, 3, HIDDEN], FP8, isOutput=False)
    wout_in = nc.declare_dram_parameter("WoutRep", [P, 2, P], BF16, isOutput=False)
    eb_in = nc.declare_dram_parameter("Ebias", [P, 2, n_steps], F32, isOutput=False)
    hb_in = nc.declare_dram_parameter("Hbias", [P, 2, 3], F32, isOutput=False)
    out_ext = nc.declare_dram_parameter("outT", [NGROUPS, P, NB], F32R, isOutput=True)

    s_t = _cached["tables"]["s_t"]

    with tile.TileContext(nc) as tc:
        with ExitStack() as ctx:
            wpool = ctx.enter_context(tc.tile_pool(name="weights", bufs=1))
            zpool = ctx.enter_context(tc.tile_pool(name="zbufs", bufs=1))
            hpool = ctx.enter_context(tc.tile_pool(name="acts", bufs=3))
            wstream = ctx.enter_context(tc.tile_pool(name="wstream", bufs=3))
            ppool = ctx.enter_context(tc.tile_pool(name="psum", bufs=3, space="PSUM"))

            stateT = wpool.tile([P, B_CORE], F32R, tag="stateT")
            w0s = wpool.tile([P, HIDDEN], F32R, tag="w0s")
            wh = wpool.tile([P, 2, 3, HIDDEN], FP8, tag="wh")
            wout = wpool.tile([P, 2, P], BF16, tag="wout")
            eb = wpool.tile([P, 2, n_steps], F32, tag="eb")
            hb = wpool.tile([P, 2, 3], F32, tag="hb")

            for c in range(NCHUNKS):
                nc.sync.dma_start(
                    stateT[:, c * NC : (c + 1) * NC],
                    state_in[:, c * NC : (c + 1) * NC],
                )
            nc.sync.dma_start(w0s[:], w0s_in[:])
            nc.sync.dma_start(wh[:], wh_in[:])
            nc.sync.dma_start(wout[:], wout_in[:])
            nc.sync.dma_start(eb[:], eb_in[:])
            nc.sync.dma_start(hb[:], hb_in[:])

            zt = [
                [zpool.tile([P, NB], F32R, tag=f"z{p}_{g}", name=f"z{p}_{g}")
                 for g in range(NGROUPS)]
                for p in range(2)
            ]
            for p in range(2):
                for g in range(NGROUPS):
                    nc.sync.dma_start(zt[p][g][:], zin_in[g])

            for t in range(n_steps):
                zc, zn = zt[t % 2], zt[(t + 1) % 2]
                w0a_t = wstream.tile([P, HIDDEN], F32R, tag="w0a_t", name="w0a_t")
                nc.sync.dma_start(w0a_t[:], w0a_in[t])

                # layer-major across each group's two chunks so the PE
                # streams chunk c1's matmuls while c0's drains run
                for g in range(NGROUPS):
                    # ---- layer 0: state MM + row-tiled z MM
                    # j-outer so one w0s LDWEIGHTS covers both chunks
                    pl0 = {}
                    for j in range(2):
                        jsl = slice(j * P, (j + 1) * P)
                        for ci in range(2):
                            c = 2 * g + ci
                            p0 = ppool.tile([P, NC], F32, tag="big", name="p0")
                            for s in range(2):
                                nc.tensor.matmul(
                                    p0[:, s * NB : (s + 1) * NB], w0s[:, jsl],
                                    stateT[:, c * NC + s * NB : c * NC + (s + 1) * NB],
                                    start=True, stop=False)
                            pl0[(ci, j)] = p0
                        for ci in range(2):
                            for s in range(2):
                                bp = 32 * (2 * ci + s)
                                nc.tensor.matmul(
                                    pl0[(ci, j)][:, s * NB : (s + 1) * NB],
                                    w0a_t[bp : bp + 32, jsl],
                                    zc[g][bp : bp + 32, :],
                                    start=False, stop=True,
                                    tile_position=(bp, 0))
                    hprev = {}
                    for ci in range(2):
                        h0 = hpool.tile([P, 2, NC], FP8, tag="h0", name="h0")
                        for j in range(2):
                            bias = eb[:, j, t : t + 1]
                            if j == 1:
                                nc.scalar.activation(
                                    h0[:, j, :], pl0[(ci, j)][:], AF.Relu,
                                    bias=bias)
                            else:
                                nc.vector.tensor_scalar(
                                    h0[:, j, :], pl0[(ci, j)][:], bias, 0.0,
                                    ALU.add, ALU.max)
                        hprev[ci] = h0

                    # ---- hidden layers (fp8 DoubleRow, K=256)
                    for l in range(3):
                        pl = {}
                        for j in range(2):
                            jsl = slice(j * P, (j + 1) * P)
                            for ci in range(2):
                                p = ppool.tile([P, NC], F32, tag="big", name="pl")
                                for s in range(2):
                                    ssl = slice(s * NB, (s + 1) * NB)
                                    nc.tensor.matmul(
                                        p[:, ssl],
                                        wh[:, :, l, jsl],
                                        hprev[ci][:, :, ssl],
                                        start=True, stop=True, perf_mode=DR)
                                pl[(ci, j)] = p
                        hnext = {}
                        for ci in range(2):
                            c = 2 * g + ci
                            if l < 2:
                                hn = hpool.tile(
                                    [P, 2, NC], FP8, tag=f"h{l + 1}",
                                    name=f"h{l + 1}")
                            else:
                                hn = hpool.tile(
                                    [P, 2, NC], BF16, tag="h3", name="h3")
                            for j in range(2):
                                bias = hb[:, j, l : l + 1]
                                on_act = (j == 1) or (l == 0 and c % 2 == 1)
                                if on_act:
                                    nc.scalar.activation(
                                        hn[:, j, :], pl[(ci, j)][:], AF.Relu,
                                        bias=bias)
                                else:
                                    nc.vector.tensor_scalar(
                                        hn[:, j, :], pl[(ci, j)][:], bias, 0.0,
                                        ALU.add, ALU.max)
                            hnext[ci] = hn
                        hprev = hnext

                    # ---- out MM (col-tiled, 4 blocks stacked) + z-update
                    po = ppool.tile([P, NB], F32, tag="out", name="po", bufs=2)
                    for b in range(4):
                        rhs3 = hprev[b // 2]
                        ssl = slice((b % 2) * NB, (b % 2 + 1) * NB)
                        for cc in range(2):
                            nc.tensor.matmul(
                                po[32 * b : 32 * b + 32, :],
                                wout[:, cc, 32 * b : 32 * b + 32],
                                rhs3[:, cc, ssl],
                                start=(cc == 0), stop=(cc == 1),
                                tile_position=(0, 32 * b))
                    nc.vector.scalar_tensor_tensor(
                        zn[g][:], po[:], float(s_t[t]), zc[g][:],
                        ALU.mult, ALU.add)

            zfin = zt[n_steps % 2]
            for g in range(NGROUPS):
                nc.sync.dma_start(out_ext[g], zfin[g][:])

    nc.compile()
    return nc


def _prepare(state, init_noise, W0, b0, Wh, bh, Wout, bout):
    state = np.ascontiguousarray(np.asarray(state, np.float32))
    init_noise = np.ascontiguousarray(np.asarray(init_noise, np.float32))
    Wh_np = np.asarray(Wh, np.float32)
    bh_np = np.asarray(bh, np.float32)
    Wout_np = np.asarray(Wout, np.float32)

    tables = _host_tables(
        np.asarray(W0, np.float32), np.asarray(b0, np.float32),
        Wh_np, bh_np, Wout_np, np.asarray(bout, np.float32),
    )
    tables["Wout_f"] = Wout_np
    _cached["tables"] = tables

    n_steps = int(os.environ.get("DPH_KERNEL_STEPS", N_STEPS))

    # fp8 hidden weights with output-aware rounding (sample = first rows)
    S0 = int(os.environ.get("DPH_OPT_SAMPLE", 192))
    Whq, bcorr = _quantize_hidden(
        tables, Wh_np, bh_np, state[:S0], init_noise[:S0])
    _cached["whq"] = Whq
    _cached["bcorr"] = bcorr

    if _cached.get("nc_steps") != n_steps:
        _cached["nc"] = _build_program(n_steps)
        _cached["nc_steps"] = n_steps
    nc = _cached["nc"]

    # ---- device layouts (shared across cores)
    W0A = tables["W0A"]
    # replicated z-weight table [50, 128, 256]
    w0a_rep = np.ascontiguousarray(
        np.tile(W0A[:ACTION_DIM, :n_steps].transpose(1, 0, 2), (1, 4, 1)),
        np.float32)
    # e' bias row -> [128, 2, n_steps]
    e_bias = np.ascontiguousarray(
        W0A[ACTION_DIM, :n_steps].T.reshape(2, P, n_steps).transpose(1, 0, 2),
        np.float32)
    # hidden DoubleRow weights [128, 2, 3, 256]
    whdr = np.zeros((P, 2, 3, HIDDEN), E4NP)
    for l in range(3):
        q = Whq[l]                                   # [256, 256] fp8
        whdr[:, 0, l, :] = q[0:P]
        whdr[:, 1, l, :] = q[P : 2 * P]
    # drain biases [128, 2, 3] in psum units
    hbias = np.zeros((P, 2, 3), np.float32)
    for l in range(3):
        v = ALPHA[l + 1] * bh_np[l] + bcorr[l]
        hbias[:, 0, l] = v[0:P]
        hbias[:, 1, l] = v[P : 2 * P]
    # out weights: Wout/ALPHA3, col-replicated [128, 2, 128] bf16
    wout_eff = (Wout_np / ALPHA[3]).astype(np.float32)
    wout_rep = np.ascontiguousarray(
        np.stack([np.tile(wout_eff[0:P], (1, 4)),
                  np.tile(wout_eff[P : 2 * P], (1, 4))], axis=1)).astype(BF16NP)

    in_maps = []
    for c in range(N_CORES):
        rows = slice(c * B_CORE, (c + 1) * B_CORE)
        st = state[rows]                             # [4096, 128]
        nz = init_noise[rows]                        # [4096, 32]
        # z stacks: [group][32b+k, n] = noise[(4g+b)*512 + n, k]
        zs = np.ascontiguousarray(
            nz.reshape(NGROUPS, 4, NB, ACTION_DIM)
              .transpose(0, 1, 3, 2)
              .reshape(NGROUPS, P, NB), np.float32)
        in_maps.append(
            {
                "stateT": np.ascontiguousarray(st.T),
                "zinit": zs,
                "W0s": tables["W0s"],
                "W0Arep": w0a_rep,
                "WhDR": whdr,
                "WoutRep": wout_rep,
                "Ebias": e_bias,
                "Hbias": hbias,
            }
        )

    _cached["in_maps"] = in_maps
    return nc, in_maps, tables


def _assemble_output(tables, per_core_outT):
    g50 = np.float32(tables["g_final"])
    beta50 = tables["beta_final"].astype(np.float32)
    n = len(per_core_outT)
    out = np.empty((n * B_CORE, ACTION_DIM), np.float32)
    for c in range(n):
        rows = slice(c * B_CORE, (c + 1) * B_CORE)
        zf = np.asarray(per_core_outT[c], np.float32)  # [2, 128, 512]
        zf = zf.reshape(NGROUPS, 4, ACTION_DIM, NB).transpose(0, 1, 3, 2)
        out[rows] = g50 * zf.reshape(B_CORE, ACTION_DIM) + beta50
    return out


def kernel(state, init_noise, W0, b0, Wh, bh, Wout, bout):
    from concourse.bass_utils import run_bass_kernel_spmd

    nc, in_maps, tables = _prepare(
        state, init_noise, W0, b0, Wh, bh, Wout, bout)
    trace = bool(int(os.environ.get("DPH_TRACE", "0")))
    res = run_bass_kernel_spmd(
        nc, in_maps, core_ids=list(range(N_CORES)), trace=trace
    )
    _cached["last_results"] = res
    return _assemble_output(tables, [res.results[c]["outT"] for c in range(N_CORES)])


if __name__ == "__main__":
    _c = np.load("/root/problem/ref_cache.npz")
    inputs = {k: _c[k] for k in _c.files if k != "expected"}
    exp = _c["expected"]
    if os.environ.get("DPH_SIM", "0") == "1":
        # CoreSim single-core check (core 0 -> rows 0:4096)
        from concourse.bass_interp import CoreSim

        nc, in_maps, tables = _prepare(**inputs)
        sim = CoreSim(nc, trace=False)
        for k, v in in_maps[0].items():
            sim.tensor(k)[:] = v
        sim.simulate()
        out0 = _assemble_output(tables, [np.asarray(sim.tensor("outT"))])
        d = np.linalg.norm(out0 - exp[:B_CORE]) / np.linalg.norm(exp[:B_CORE])
        print(f"CoreSim core0 L2 relative error: {d:.4e}")
    else:
        got = kernel(**inputs)
        d = np.linalg.norm(got - exp) / np.linalg.norm(exp)
        print(f"L2 relative error: {d:.4e}")
